# revision 1
# baseline (speedup 1.0000x reference)
"""Trainium2 Bass kernel for nn_DownstreamModel (2-layer GraphSAGE + MLP head).

Sharding: nodes (and their incident edges, bucketed by dst) are split across
8 NeuronCores in contiguous ranges of 6250 nodes. Weights are replicated.
h1 is AllGathered between the two SAGE layers; pooled per-graph features are
AllReduced before the (replicated) MLP head.

Segment-sum (the message aggregation) is computed as a matmul with on-device
built one-hot selection tiles: for each 128-edge tile with destinations inside
one 128-node block, PSUM accumulates  P^T.T @ M  where P^T[e, j] = (dst[e]==j)
and M = gathered source features (batched indirect DMA).
"""

import math
from dataclasses import dataclass, field

import numpy as np

import concourse.bass as bass
import concourse.bacc as bacc
import concourse.mybir as mybir
import concourse.tile as tile
from concourse.bass import IndirectOffsetOnAxis

# ---------------- problem constants (hardcoded, per harness contract) -------
N_NODES = 50000
N_EDGES = 1000000
N_GRAPHS = 64
F_IN = 113
H = 512
Z_DIM = 518
NCORES = 8

F_PAD = 128          # x gather-table row width (col F_IN holds 1.0 for degree)
P = 128              # partitions / block size

LAST_RUN_INFO: dict = {}


@dataclass
class Cfg:
    n_nodes: int = N_NODES
    n_graphs: int = N_GRAPHS
    f_in: int = F_IN
    h: int = H
    z_dim: int = Z_DIM
    ncores: int = NCORES
    t_blk: int = 22            # edge tiles per node block (uniform, data-dep)
    t_half: int = 11           # edge tiles per (block, src-half)
    gather_chunk: int = 8      # L2 gather batching (tiles per dma_gather call)
    dt_l1: mybir.dt = mybir.dt.bfloat16  # x table / L1 messages / one-hot dtype
    dt_l2: mybir.dt = mybir.dt.bfloat16  # h1 table / L2 messages / one-hot dtype
    dt_w: mybir.dt = mybir.dt.float32r   # dense matmul operand dtype (weights/acts)
    n_queues: int = 4          # SWDGE queues for gather desc-gen parallelism
    trace: bool = False
    debug_outputs: bool = False
    use_collectives: bool = True
    stop_after: str = "all"   # l1 | l2 | all

    @property
    def nc_nodes(self):
        return self.n_nodes // self.ncores

    @property
    def nblocks(self):
        return (self.nc_nodes + P - 1) // P

    @property
    def h_chunks(self):
        return self.h // P


# ------------------------- host-side shard prep (int only) ------------------
def prep_shards(edge_index: np.ndarray, cfg: Cfg):
    """Bucket edges by (dst core, dst block, src half); pad each bucket to a
    uniform number of 128-edge tiles (t_half per half). Returns per-core
    int16 gather-index arrays (dma_gather layout: value for edge-column ct,
    partition p lands at [p % 16, ct * 8 + p // 16], replicated to 128
    partitions) and per-edge dst-in-block floats [128, nt]. Integer-only."""
    src = np.ascontiguousarray(edge_index[0]).astype(np.int64)
    dst = np.ascontiguousarray(edge_index[1]).astype(np.int64)
    ncn, nb, ncores = cfg.nc_nodes, cfg.nblocks, cfg.ncores
    half_sz = cfg.n_nodes // 2

    order = np.argsort(dst, kind="stable")
    src_s = src[order]
    dst_s = dst[order]

    core = dst_s // ncn
    local = dst_s - core * ncn
    blk = local // P
    assert ncn % 2 == 0
    hcn = ncn // 2
    s_core = src_s // ncn
    s_local = src_s - s_core * ncn
    half = s_local // hcn                       # src chunk (0/1)
    key = (core * nb + blk) * 2 + half
    # edges sorted by dst, need stable sort by half within each (core, blk)
    order2 = np.argsort(key, kind="stable")
    src_s, dst_s, core, local, blk, half, key, s_core, s_local = (
        a[order2]
        for a in (src_s, dst_s, core, local, blk, half, key, s_core, s_local)
    )
    counts = np.bincount(key, minlength=ncores * nb * 2)
    t_half = max(1, int(math.ceil(counts.max() / P)))
    cfg.t_half = t_half
    cfg.t_blk = 2 * t_half
    nt = nb * 2 * t_half

    starts = np.zeros(ncores * nb * 2 + 1, np.int64)
    np.cumsum(counts, out=starts[1:])
    rank = np.arange(len(dst_s)) - starts[key]
    ct = (blk * 2 + half) * t_half + rank // P          # edge column within core
    pp = rank % P
    slot = ct * P + pp
    src16 = (s_core * hcn + s_local - half * hcn).astype(np.int16)

    idxs, dstbls = [], []
    for c in range(ncores):
        m = core == c
        s_arr = np.zeros(nt * P, np.int16)               # pad -> row 0
        d_arr = np.full(nt * P, -1.0, np.float32)        # pad -> -1 (no match)
        s_arr[slot[m]] = src16[m]
        d_arr[slot[m]] = (local[m] - blk[m] * P).astype(np.float32)
        # dma_gather index staging: [16, nt*8] -> tile to [128, nt*8]
        st = s_arr.reshape(nt, 8, 16).transpose(2, 0, 1).reshape(16, nt * 8)
        idxs.append(np.ascontiguousarray(np.tile(st, (8, 1))))
        dstbls.append(np.ascontiguousarray(d_arr.reshape(nt, P).T))  # [128, nt]
    return idxs, dstbls


def prep_inputs(x, edge_index, batch_ids, z, params: dict, cfg: Cfg):
    """Build the 8 per-core input maps (layout/replication staging only)."""
    ncn, nb, ncores = cfg.nc_nodes, cfg.nblocks, cfg.ncores
    f_in, h, zd = cfg.f_in, cfg.h, cfg.z_dim
    hc = cfg.h_chunks
    fz = h + zd
    fz_pad = ((fz + P - 1) // P) * P

    idx16s, dstbls = prep_shards(edge_index, cfg)

    xpad = np.zeros((cfg.n_nodes, F_PAD), np.float32)
    xpad[:, :f_in] = x
    xpad[:, f_in] = 1.0                     # degree counter column
    # permute rows to the chunked-AllGather layout:
    # row(g) = chunk*(N/2) + core(g)*(ncn/2) + local(g) % (ncn/2)
    g = np.arange(cfg.n_nodes)
    gc, gl = g // ncn, g % ncn
    hcn = ncn // 2
    ch = gl // hcn
    perm = ch * (cfg.n_nodes // 2) + gc * hcn + (gl - ch * hcn)
    xpad_p = np.empty_like(xpad)
    xpad_p[perm] = xpad
    xpad = xpad_p

    batchf_full = batch_ids.astype(np.float32)

    iota128 = np.tile(np.arange(P, dtype=np.float32)[None, :], (P, 1))
    iotagr = np.tile(np.arange(cfg.n_graphs, dtype=np.float32)[None, :], (P, 1))
    ident = np.eye(P, dtype=np.float32)

    def padT(w, rows):  # w [out, in] -> w.T padded to [rows, out]
        wt = np.zeros((rows, w.shape[0]), np.float32)
        wt[: w.shape[1], :] = np.ascontiguousarray(w.T)
        return wt

    wl1t = padT(params["Wl1"], P)           # [128, 512]
    wr1t = padT(params["Wr1"], P)
    wl2t = padT(params["Wl2"], h)           # [512, 512]
    wr2t = padT(params["Wr2"], h)
    wf1t = padT(params["Wf1"], fz_pad)      # [1152, 512]
    wf2t = padT(params["Wf2"], h)
    wpt = padT(params["Wp"], h)             # [512, 1]

    brep = lambda b: np.tile(b.astype(np.float32)[None, :], (P, 1))
    bl1r, bl2r = brep(params["bl1"]), brep(params["bl2"])
    bf1r, bf2r = brep(params["bf1"]), brep(params["bf2"])
    bpr = np.tile(params["bp"].astype(np.float32).reshape(1, 1), (P, 1))

    in_maps = []
    for c in range(ncores):
        lo = c * ncn
        xT = np.zeros((P, nb * P), np.float32)
        nrows = min(ncn, cfg.n_nodes - lo)
        xT[:f_in, :nrows] = x[lo : lo + nrows].T
        bfl = np.full(nb * P, -1.0, np.float32)
        bfl[:nrows] = batchf_full[lo : lo + nrows]
        batchf = np.ascontiguousarray(bfl.reshape(nb, P).T)  # node (b,p) at [p, b]
        in_maps.append(
            {
                "xpad": xpad,
                "xT": xT,
                "idx16": idx16s[c],
                "dstbl": dstbls[c],
                "batchf": batchf,
                "iota128": iota128,
                "iotagr": iotagr,
                "ident": ident,
                "wl1t": wl1t,
                "wr1t": wr1t,
                "wl2t": wl2t,
                "wr2t": wr2t,
                "wf1t": wf1t,
                "wf2t": wf2t,
                "wpt": wpt,
                "bl1r": bl1r,
                "bl2r": bl2r,
                "bf1r": bf1r,
                "bf2r": bf2r,
                "bpr": bpr,
                "zin": z.astype(np.float32),
            }
        )
    return in_maps


# ------------------------------- device program -----------------------------
def build_nc(cfg: Cfg):
    ncn, nb, ncores = cfg.nc_nodes, cfg.nblocks, cfg.ncores
    f_in, h, zd, ng = cfg.f_in, cfg.h, cfg.z_dim, cfg.n_graphs
    hc = cfg.h_chunks
    t_blk = cfg.t_blk
    t_half = cfg.t_half
    half_sz = cfg.n_nodes // 2
    nt = nb * t_blk
    fz = h + zd
    fz_pad = ((fz + P - 1) // P) * P
    fzc = fz_pad // P
    f32 = mybir.dt.float32
    dt1, dt2 = cfg.dt_l1, cfg.dt_l2

    nc = bacc.Bacc(
        "TRN2", target_bir_lowering=False, debug=False, num_devices=ncores,
        num_swdge_queues=cfg.n_queues,
    )
    qctr = [0]

    def next_q():
        qctr[0] += 1
        return qctr[0] % cfg.n_queues

    # ---- I/O ----
    xpad_d = nc.dram_tensor("xpad", [cfg.n_nodes, F_PAD], f32, kind="ExternalInput")
    xT_d = nc.dram_tensor("xT", [P, nb * P], f32, kind="ExternalInput")
    idx16_d = nc.dram_tensor("idx16", [P, nt * 8], mybir.dt.int16, kind="ExternalInput")
    dstbl_d = nc.dram_tensor("dstbl", [P, nt], f32, kind="ExternalInput")
    batchf_d = nc.dram_tensor("batchf", [P, nb], f32, kind="ExternalInput")
    iota128_d = nc.dram_tensor("iota128", [P, P], f32, kind="ExternalInput")
    iotagr_d = nc.dram_tensor("iotagr", [P, ng], f32, kind="ExternalInput")
    ident_d = nc.dram_tensor("ident", [P, P], f32, kind="ExternalInput")
    wl1t_d = nc.dram_tensor("wl1t", [P, h], f32, kind="ExternalInput")
    wr1t_d = nc.dram_tensor("wr1t", [P, h], f32, kind="ExternalInput")
    wl2t_d = nc.dram_tensor("wl2t", [h, h], f32, kind="ExternalInput")
    wr2t_d = nc.dram_tensor("wr2t", [h, h], f32, kind="ExternalInput")
    wf1t_d = nc.dram_tensor("wf1t", [fz_pad, h], f32, kind="ExternalInput")
    wf2t_d = nc.dram_tensor("wf2t", [h, h], f32, kind="ExternalInput")
    wpt_d = nc.dram_tensor("wpt", [h, 1], f32, kind="ExternalInput")
    bl1r_d = nc.dram_tensor("bl1r", [P, h], f32, kind="ExternalInput")
    bl2r_d = nc.dram_tensor("bl2r", [P, h], f32, kind="ExternalInput")
    bf1r_d = nc.dram_tensor("bf1r", [P, h], f32, kind="ExternalInput")
    bf2r_d = nc.dram_tensor("bf2r", [P, h], f32, kind="ExternalInput")
    bpr_d = nc.dram_tensor("bpr", [P, 1], f32, kind="ExternalInput")
    zin_d = nc.dram_tensor("zin", [ng, zd], f32, kind="ExternalInput")
    out_d = nc.dram_tensor("out", [ng, 1], f32, kind="ExternalOutput")
    if cfg.debug_outputs:
        h1s_dbg = nc.dram_tensor("h1s_dbg", [ncn, h], f32, kind="ExternalOutput")
        h1f_dbg = nc.dram_tensor("h1f_dbg", [2 * P, h], f32, kind="ExternalOutput")
        pool_dbg = nc.dram_tensor("pool_dbg", [ng, h], f32, kind="ExternalOutput")
        agg_dbg = nc.dram_tensor("agg_dbg", [P, h], f32, kind="ExternalOutput")

    dtw = cfg.dt_w

    with tile.TileContext(nc) as tc:
        with (
            tc.tile_pool(name="const", bufs=1) as cp,
            tc.tile_pool(name="dram", bufs=1, space="DRAM") as dp,
            tc.tile_pool(name="gath", bufs=2) as gp,
            tc.tile_pool(name="oh", bufs=3) as ohp,
            tc.tile_pool(name="work", bufs=2) as wp,
            tc.tile_pool(name="tp", bufs=3) as tp,
            tc.tile_pool(name="hout", bufs=2) as hp,
            tc.tile_pool(name="ps_agg", bufs=3, space="PSUM") as ps_aggp,
            tc.tile_pool(name="ps_t", bufs=2, space="PSUM") as ps_tp,
            tc.tile_pool(name="ps_o", bufs=2, space="PSUM") as ps_op,
            tc.tile_pool(name="ps_pool", bufs=1, space="PSUM") as ps_poolp,
        ):
            # ---- persistent SBUF state ----
            idx16 = cp.tile([P, nt * 8], mybir.dt.int16)
            nc.sync.dma_start(idx16[:], idx16_d[:])
            dstbl = cp.tile([P, nt], dt1)
            if dt1 == f32:
                nc.sync.dma_start(dstbl[:], dstbl_d[:])
            else:
                dstbl_f = cp.tile([P, nt], f32)
                nc.sync.dma_start(dstbl_f[:], dstbl_d[:])
                nc.vector.tensor_copy(dstbl[:], dstbl_f[:])
            batchf = cp.tile([P, nb], f32)
            nc.sync.dma_start(batchf[:], batchf_d[:])
            iota128 = cp.tile([P, P], dt1)
            if dt1 == f32:
                nc.sync.dma_start(iota128[:], iota128_d[:])
            else:
                iota_f = cp.tile([P, P], f32)
                nc.sync.dma_start(iota_f[:], iota128_d[:])
                nc.vector.tensor_copy(iota128[:], iota_f[:])
            iotagr = cp.tile([P, ng], f32)
            nc.sync.dma_start(iotagr[:], iotagr_d[:])
            ident = cp.tile([P, P], f32)
            nc.sync.dma_start(ident[:], ident_d[:])
            if dtw != f32:
                ident_w = cp.tile([P, P], dtw)
                nc.vector.tensor_copy(ident_w[:], ident[:])
            else:
                ident_w = ident
            if dt2 == f32:
                ident_b = ident
            elif dt2 == dtw:
                ident_b = ident_w
            else:
                ident_b = cp.tile([P, P], dt2)
                nc.vector.tensor_copy(ident_b[:], ident[:])

            def load_w(name, dram_ap, shape):
                if dtw == f32:
                    t = cp.tile(shape, f32, name=name)
                    nc.sync.dma_start(t[:], dram_ap)
                    return t
                t = cp.tile(shape, dtw, name=name)
                if len(shape) == 2:
                    tmp = wp.tile([P, shape[1]], f32, tag="wtmp", padded_shape=[P, h])
                    nc.sync.dma_start(tmp[:], dram_ap)
                    nc.vector.tensor_copy(t[:], tmp[:])
                else:
                    for k in range(shape[1]):
                        tmp = wp.tile([P, shape[2]], f32, tag="wtmp", padded_shape=[P, h])
                        nc.sync.dma_start(tmp[:], dram_ap[:, k, :])
                        nc.vector.tensor_copy(t[:, k, :], tmp[:])
                return t

            wl1t = load_w("wl1t", wl1t_d[:], [P, h])
            wr1t = load_w("wr1t", wr1t_d[:], [P, h])
            wl2t = load_w("wl2t", wl2t_d[:].rearrange("(k p) n -> p k n", p=P), [P, hc, h])
            wr2t = load_w("wr2t", wr2t_d[:].rearrange("(k p) n -> p k n", p=P), [P, hc, h])
            wf1t = cp.tile([P, fzc, h], f32)
            nc.sync.dma_start(wf1t[:], wf1t_d[:].rearrange("(k p) n -> p k n", p=P))
            wf2t = cp.tile([P, hc, h], f32)
            nc.sync.dma_start(wf2t[:], wf2t_d[:].rearrange("(k p) n -> p k n", p=P))
            wpt = cp.tile([P, hc, 1], f32)
            nc.sync.dma_start(wpt[:], wpt_d[:].rearrange("(k p) n -> p k n", p=P))
            bl1r = cp.tile([P, h], f32)
            nc.sync.dma_start(bl1r[:], bl1r_d[:])
            bl2r = cp.tile([P, h], f32)
            nc.sync.dma_start(bl2r[:], bl2r_d[:])
            bf1r = cp.tile([P, h], f32)
            nc.sync.dma_start(bf1r[:], bf1r_d[:])
            bf2r = cp.tile([P, h], f32)
            nc.sync.dma_start(bf2r[:], bf2r_d[:])
            bpr = cp.tile([P, 1], f32)
            nc.sync.dma_start(bpr[:], bpr_d[:])

            inv_all = cp.tile([P, nb], f32)      # 1/max(deg,1) per node

            # self-feature transpose table, SBUF-resident in matmul dtype
            xTs = cp.tile([P, nb * P], dtw)
            if dtw == f32:
                nc.sync.dma_start(xTs[:], xT_d[:])
            else:
                for c0 in range(0, nb * P, 1024):
                    c1 = min(nb * P, c0 + 1024)
                    xtmp = wp.tile([P, 1024], f32, tag="xtmp")
                    nc.sync.dma_start(xtmp[:, : c1 - c0], xT_d[:, c0:c1])
                    nc.vector.tensor_copy(xTs[:, c0:c1], xtmp[:, : c1 - c0])

            # ---- internal DRAM ----
            hcn = ncn // 2
            h1_shard = dp.tile([ncn, h], dt2)                 # AG input (local)
            h1_fa = dp.tile([hcn * ncores, h], dt2, addr_space="Shared")
            h1_fb = dp.tile([hcn * ncores, h], dt2, addr_space="Shared")
            pool_in = dp.tile([ng, h], f32)
            pool_out = dp.tile([ng, h], f32, addr_space="Shared")

            if dt1 == f32:
                xgath = xpad_d
            else:
                xgath = dp.tile([cfg.n_nodes, F_PAD], dt1)
                nc.gpsimd.dma_start(xgath[:], xpad_d[:])  # SWDGE f32->bf16 cast

            def rows_of(b):
                return min(P, ncn - b * P)

            # =================== Layer 1 ===================
            for b in range(nb):
                ps_agg = ps_aggp.tile([P, F_PAD], f32, tag="agg", padded_shape=[P, h])
                for hf in range(2):
                    ct0 = b * t_blk + hf * t_half
                    m1 = gp.tile([P, t_half, F_PAD], dt1, tag="m1", bufs=3)
                    for g0 in range(0, t_half, 8):
                        gw = min(8, t_half - g0)
                        nc.gpsimd.dma_gather(
                            out_ap=m1[:, g0 : g0 + gw, :],
                            in_ap=xgath[hf * half_sz : (hf + 1) * half_sz, :],
                            idxs_ap=idx16[:, (ct0 + g0) * 8 : (ct0 + g0 + gw) * 8],
                            num_idxs=gw * P,
                            num_idxs_reg=gw * P,
                            elem_size=F_PAD,
                            queue_num=next_q(),
                        )
                    oh = ohp.tile([P, t_half, P], dt1, tag="oh")
                    nc.any.tensor_tensor(
                        out=oh[:],
                        in0=dstbl[:, ct0 : ct0 + t_half]
                        .unsqueeze(2)
                        .to_broadcast([P, t_half, P]),
                        in1=iota128[:].unsqueeze(1).to_broadcast([P, t_half, P]),
                        op=mybir.AluOpType.is_equal,
                    )
                    for t in range(t_half):
                        nc.tensor.matmul(
                            ps_agg[:],
                            lhsT=oh[:, t, :],
                            rhs=m1[:, t, :],
                            start=(hf == 0 and t == 0),
                            stop=(hf == 1 and t == t_half - 1),
                        )
                # degree -> 1/max(cnt,1)
                cnt = wp.tile([P, 1], f32, tag="cnt")
                nc.vector.tensor_scalar_max(cnt[:], ps_agg[:, f_in : f_in + 1], 1.0)
                nc.vector.reciprocal(inv_all[:, b : b + 1], cnt[:])
                agg = wp.tile([P, P], dtw, tag="agg_sb")
                nc.vector.tensor_scalar_mul(
                    agg[:], ps_agg[:, :P], inv_all[:, b : b + 1]
                )
                # transpose agg -> [f, n]
                ps_t = ps_tp.tile([P, P], dtw, tag="pst")
                nc.tensor.transpose(ps_t[:], agg[:], ident_w[:])
                aggT = tp.tile([P, P], dtw, tag="aggT")
                nc.vector.tensor_copy(aggT[:], ps_t[:])
                ps_o = ps_op.tile([P, h], f32, tag="pso")
                nc.tensor.matmul(
                    ps_o[:], lhsT=aggT[:], rhs=wl1t[:],
                    start=True, stop=False,
                )
                nc.tensor.matmul(
                    ps_o[:], lhsT=xTs[:, b * P : (b + 1) * P], rhs=wr1t[:],
                    start=False, stop=True,
                )
                o_sb = wp.tile([P, h], f32, tag="o_sb")
                nc.vector.tensor_tensor(
                    out=o_sb[:], in0=ps_o[:], in1=bl1r[:], op=mybir.AluOpType.add
                )
                # row L2 norm -> scale
                ssq = wp.tile([P, 1], f32, tag="ssq")
                trash = wp.tile([P, h], f32, tag="trash", bufs=1)
                nc.scalar.activation(
                    trash[:], o_sb[:], mybir.ActivationFunctionType.Square,
                    accum_out=ssq[:],
                )
                nrm = wp.tile([P, 1], f32, tag="nrm")
                nc.scalar.sqrt(nrm[:], ssq[:])
                nc.vector.tensor_scalar_max(nrm[:], nrm[:], 1e-12)
                rinv = wp.tile([P, 1], f32, tag="rinv")
                nc.vector.reciprocal(rinv[:], nrm[:])
                h1b = hp.tile([P, h], dt2, tag="h1b")
                nc.scalar.activation(
                    h1b[:], o_sb[:], mybir.ActivationFunctionType.Relu,
                    scale=rinv[:],
                )
                r = rows_of(b)
                nc.sync.dma_start(h1_shard[b * P : b * P + r, :], h1b[:r, :])

            # =================== AllGather h1 (2 chunks) ===================
            if ncores > 1 and cfg.use_collectives:
                nc.gpsimd.collective_compute(
                    "AllGather",
                    mybir.AluOpType.bypass,
                    replica_groups=[list(range(ncores))],
                    ins=[h1_shard[:hcn, :].opt()],
                    outs=[h1_fa.opt()],
                )
                nc.gpsimd.collective_compute(
                    "AllGather",
                    mybir.AluOpType.bypass,
                    replica_groups=[list(range(ncores))],
                    ins=[h1_shard[hcn:, :].opt()],
                    outs=[h1_fb.opt()],
                )
            else:
                nc.sync.dma_start(h1_fa[:hcn, :], h1_shard[:hcn, :])
                nc.sync.dma_start(h1_fb[:hcn, :], h1_shard[hcn:, :])

            if cfg.debug_outputs:
                nc.sync.dma_start(h1s_dbg[:], h1_shard[:])
                nc.sync.dma_start(h1f_dbg[:], h1_fa[: 2 * P, :])

            # =================== Layer 2 (+ fused graph pooling) =============
            ps_pool = ps_poolp.tile([ng, h], f32)
            l2_blocks = nb if cfg.stop_after != "l1" else 0
            for b in range(l2_blocks):
                ps_agg = ps_aggp.tile([P, h], f32, tag="agg")
                t = 0
                for hf in range(2):
                    ct0h = b * t_blk + hf * t_half
                    oh = ohp.tile([P, t_half, P], dt2, tag="oh")
                    nc.any.tensor_tensor(
                        out=oh[:],
                        in0=dstbl[:, ct0h : ct0h + t_half]
                        .unsqueeze(2)
                        .to_broadcast([P, t_half, P]),
                        in1=iota128[:].unsqueeze(1).to_broadcast([P, t_half, P]),
                        op=mybir.AluOpType.is_equal,
                    )
                    for t0h in range(0, t_half, cfg.gather_chunk):
                        tw = min(cfg.gather_chunk, t_half - t0h)
                        ct0 = ct0h + t0h
                        m2 = gp.tile(
                            [P, tw, h], dt2, tag="m2", bufs=4,
                            padded_shape=[P, cfg.gather_chunk, h],
                        )
                        nc.gpsimd.dma_gather(
                            out_ap=m2[:],
                            in_ap=(h1_fa if hf == 0 else h1_fb)[:],
                            idxs_ap=idx16[:, ct0 * 8 : (ct0 + tw) * 8],
                            num_idxs=tw * P,
                            num_idxs_reg=tw * P,
                            elem_size=h,
                            queue_num=next_q(),
                        )
                        for j in range(tw):
                            nc.tensor.matmul(
                                ps_agg[:],
                                lhsT=oh[:, t0h + j, :],
                                rhs=m2[:, j, :],
                                start=(t == 0),
                                stop=(t == t_blk - 1),
                            )
                            t += 1
                agg2 = wp.tile([P, h], dtw, tag="agg2_sb")
                nc.vector.tensor_scalar_mul(
                    agg2[:], ps_agg[:], inv_all[:, b : b + 1]
                )
                if cfg.debug_outputs and b == 0:
                    nc.sync.dma_start(agg_dbg[:], agg2[:])
                # self features (from local shard; padded rows zeroed)
                h1self = wp.tile([P, h], dt2, tag="h1self")
                r = rows_of(b)
                if r < P:
                    nc.gpsimd.memset(h1self[:], 0.0)
                nc.sync.dma_start(h1self[:r, :], h1_shard[b * P : b * P + r, :])
                ps_o = ps_op.tile([P, h], f32, tag="pso")
                for k in range(hc):
                    ps_t = ps_tp.tile([P, P], dtw, tag="pst")
                    nc.tensor.transpose(
                        ps_t[:], agg2[:, k * P : (k + 1) * P], ident_w[:]
                    )
                    a2T = tp.tile([P, P], dtw, tag="a2T")
                    nc.vector.tensor_copy(a2T[:], ps_t[:])
                    nc.tensor.matmul(
                        ps_o[:],
                        lhsT=a2T[:],
                        rhs=wl2t[:, k, :],
                        start=(k == 0),
                        stop=False,
                    )
                for k in range(hc):
                    ps_t = ps_tp.tile([P, P], dt2, tag="pst")
                    nc.tensor.transpose(
                        ps_t[:], h1self[:, k * P : (k + 1) * P], ident_b[:]
                    )
                    h1T = tp.tile([P, P], dtw, tag="h1T")
                    nc.vector.tensor_copy(h1T[:], ps_t[:])
                    nc.tensor.matmul(
                        ps_o[:],
                        lhsT=h1T[:],
                        rhs=wr2t[:, k, :],
                        start=False,
                        stop=(k == hc - 1),
                    )
                o_sb = wp.tile([P, h], f32, tag="o_sb")
                nc.vector.tensor_tensor(
                    out=o_sb[:], in0=ps_o[:], in1=bl2r[:], op=mybir.AluOpType.add
                )
                ssq = wp.tile([P, 1], f32, tag="ssq")
                trash = wp.tile([P, h], f32, tag="trash", bufs=1)
                nc.scalar.activation(
                    trash[:], o_sb[:], mybir.ActivationFunctionType.Square,
                    accum_out=ssq[:],
                )
                nrm = wp.tile([P, 1], f32, tag="nrm")
                nc.scalar.sqrt(nrm[:], ssq[:])
                nc.vector.tensor_scalar_max(nrm[:], nrm[:], 1e-12)
                rinv = wp.tile([P, 1], f32, tag="rinv")
                nc.vector.reciprocal(rinv[:], nrm[:])
                h2b = hp.tile([P, h], f32, tag="h2b")
                nc.scalar.activation(
                    h2b[:], o_sb[:], mybir.ActivationFunctionType.Relu,
                    scale=rinv[:],
                )
                # graph pooling: ps_pool[g,:] += sum_{n: batch(n)==g} h2[n,:]
                G = ohp.tile([P, ng], f32, tag="G")
                nc.vector.tensor_tensor(
                    out=G[:],
                    in0=batchf[:, b : b + 1].to_broadcast([P, ng]),
                    in1=iotagr[:],
                    op=mybir.AluOpType.is_equal,
                )
                nc.tensor.matmul(
                    ps_pool[:],
                    lhsT=G[:],
                    rhs=h2b[:],
                    start=(b == 0),
                    stop=(b == nb - 1),
                )

            # =================== pool AllReduce ===================
            if cfg.stop_after == "l1":
                nc.vector.memset(ps_pool[:], 0.0)
            pool_sb = wp.tile([ng, h], f32, tag="pool_sb")
            nc.vector.tensor_copy(pool_sb[:], ps_pool[:])
            nc.sync.dma_start(pool_in[:], pool_sb[:])
            if ncores > 1 and cfg.use_collectives:
                nc.gpsimd.collective_compute(
                    "AllReduce",
                    mybir.AluOpType.add,
                    replica_groups=[list(range(ncores))],
                    ins=[pool_in.opt()],
                    outs=[pool_out.opt()],
                )
            else:
                nc.sync.dma_start(pool_out[:], pool_in[:])

            if cfg.debug_outputs:
                pdb_sb = wp.tile([ng, h], f32, tag="pdb_sb")
                nc.sync.dma_start(pdb_sb[:], pool_out[:])
                nc.sync.dma_start(pool_dbg[:], pdb_sb[:])

            # =================== MLP head (replicated) ===================
            f_sb = cp.tile([ng, fz_pad], f32)
            if fz_pad > fz:
                nc.gpsimd.memset(f_sb[:, fz:], 0.0)
            nc.sync.dma_start(f_sb[:, :h], pool_out[:])
            nc.sync.dma_start(f_sb[:, h : h + zd], zin_d[:])

            def dense_small(in_sb, w_sb, nchunks, n_out, bias, relu):
                ps = ps_op.tile([ng, n_out], f32, tag="pso")
                for k in range(nchunks):
                    ps_t = ps_tp.tile([P, ng], f32, tag="pst")
                    nc.tensor.transpose(
                        ps_t[:], in_sb[:, k * P : (k + 1) * P], ident[:ng, :ng]
                    )
                    fT = tp.tile([P, ng], f32, tag="fT")
                    nc.vector.tensor_copy(fT[:], ps_t[:])
                    nc.tensor.matmul(
                        ps[:],
                        lhsT=fT[:],
                        rhs=w_sb[:, k, :],
                        start=(k == 0),
                        stop=(k == nchunks - 1),
                    )
                g_sb = cp.tile([ng, n_out], f32, name=f"g_{relu}_{n_out}_{nchunks}")
                if bias is not None:
                    nc.vector.tensor_tensor(
                        out=g_sb[:], in0=ps[:], in1=bias[:ng, :n_out],
                        op=mybir.AluOpType.add,
                    )
                else:
                    nc.vector.tensor_copy(g_sb[:], ps[:])
                if relu:
                    nc.scalar.activation(
                        g_sb[:], g_sb[:], mybir.ActivationFunctionType.Relu
                    )
                return g_sb

            g1 = dense_small(f_sb, wf1t, fzc, h, bf1r, True)
            g2 = dense_small(g1, wf2t, hc, h, bf2r, True)
            g3 = dense_small(g2, wpt, hc, 1, bpr, False)
            res = cp.tile([ng, 1], f32)
            nc.scalar.activation(
                res[:], g3[:], mybir.ActivationFunctionType.Sigmoid
            )
            nc.scalar.activation(res[:], res[:], mybir.ActivationFunctionType.Ln)
            nc.scalar.mul(res[:], res[:], -1.0)
            nc.sync.dma_start(out_d[:], res[:])

    nc.compile()
    return nc


# ------------------------------- entry point --------------------------------
def kernel(x, edge_index, batch_ids, z, Wl1, bl1, Wr1, Wl2, bl2, Wr2,
           Wf1, bf1, Wf2, bf2, Wp, bp, _cfg: Cfg | None = None):
    import os

    from concourse.bass_utils import run_bass_kernel_spmd

    cfg = _cfg or Cfg()
    params = dict(Wl1=Wl1, bl1=bl1, Wr1=Wr1, Wl2=Wl2, bl2=bl2, Wr2=Wr2,
                  Wf1=Wf1, bf1=bf1, Wf2=Wf2, bf2=bf2, Wp=Wp, bp=bp)
    x = np.asarray(x, np.float32)
    z = np.asarray(z, np.float32)
    edge_index = np.asarray(edge_index)
    batch_ids = np.asarray(batch_ids)
    params = {k: np.asarray(v, np.float32) for k, v in params.items()}

    in_maps = prep_inputs(x, edge_index, batch_ids, z, params, cfg)
    nc = build_nc(cfg)
    res = run_bass_kernel_spmd(
        nc,
        in_maps,
        core_ids=list(range(cfg.ncores)),
        trace=cfg.trace or bool(os.environ.get("BASS_TRACE")),
    )
    LAST_RUN_INFO["exec_time_ns"] = res.exec_time_ns
    LAST_RUN_INFO["res"] = res
    LAST_RUN_INFO["results"] = res.results if cfg.debug_outputs else None
    return np.asarray(res.results[0]["out"], np.float32)



# revision 13
# speedup vs baseline: 1.5744x; 1.5744x over previous
"""Trainium2 Bass kernel for nn_DownstreamModel (2-layer GraphSAGE + MLP head).

Sharding: nodes (and incident edges, bucketed by dst) are split across 8
NeuronCores in contiguous ranges of 6250 nodes; weights replicated; the
per-graph pooled features are AllReduced before the replicated MLP head.

Key structure (v2):
- Message aggregation = one-hot matmul: PSUM accumulates oh^T @ M where
  oh[e, j] = (dst_e == j) and M = gathered source rows (SWDGE dma_gather).
- W-early transform: because mean-aggregation is linear, layer 2 computes
  agg(h1 @ Wl2^T) instead of agg(h1) @ Wl2^T. At the end of layer 1 each
  core computes h1W = h1 @ Wl2^T (AllGathered in fp8 as the L2 gather
  table) and h1Wrb = h1 @ Wr2^T + bl2 (kept local). Layer 2 then has no
  dense matmuls or transposes at all.
- fp8(e4m3) for both gather tables + one-hot tiles; message matmuls run
  in fp8 DoubleRow mode (2 edge-tiles contracted per instruction).
- Gathers are batched per (block-pair, src-half) to amortize the ~1us
  SWDGE fixed overhead per call.
"""

import math
from dataclasses import dataclass

import numpy as np
import ml_dtypes

import concourse.bass as bass
import concourse.bacc as bacc
import concourse.mybir as mybir
import concourse.tile as tile

# ---------------- problem constants (hardcoded, per harness contract) -------
N_NODES = 50000
N_EDGES = 1000000
N_GRAPHS = 64
F_IN = 113
H = 512
Z_DIM = 518
NCORES = 8

P = 128

BF16 = ml_dtypes.bfloat16
FP8 = ml_dtypes.float8_e4m3   # TRN FP8_EXP4-compatible in +-240

LAST_RUN_INFO: dict = {}


@dataclass
class Cfg:
    n_nodes: int = N_NODES
    n_graphs: int = N_GRAPHS
    f_in: int = F_IN
    h: int = H
    z_dim: int = Z_DIM
    ncores: int = NCORES
    t_half: int = 11           # edge tiles per (block, src-half); set from data
    n_queues: int = 4
    use_double_row: bool = True
    dt_msg: mybir.dt = mybir.dt.float8e4   # gather tables + oh + msg matmul
    gather_tiles_max: int = 32 # max 128-edge tiles per dma_gather call
    trace: bool = False
    debug_outputs: bool = False

    @property
    def f_pad(self):
        # x gather-row elements; row bytes must be a multiple of 256
        return 256 if mybir.dt.size(self.dt_msg) == 1 else 128

    @property
    def nc_nodes(self):
        return self.n_nodes // self.ncores

    @property
    def nblocks(self):
        return (self.nc_nodes + P - 1) // P

    @property
    def npairs(self):
        return (self.nblocks + 1) // 2

    @property
    def h_chunks(self):
        return self.h // P


def _pair_members(p: int, nb: int) -> list[int]:
    return [b for b in (2 * p, 2 * p + 1) if b < nb]


def _group_ct0(p: int, h: int, nb: int, t_half: int) -> tuple[int, int]:
    """ct range start and tile count for gather group (pair p, src half h)."""
    w = len(_pair_members(p, nb))
    base_q = 4 * p  # all earlier pairs are full (2 members x 2 halves)
    q = base_q + h * w
    return q * t_half, w * t_half


# ------------------------- host-side shard prep -----------------------------
def prep_shards(edge_index: np.ndarray, cfg: Cfg):
    """Bucket edges by (dst core, block pair, src half, pair member); pad each
    bucket to t_half 128-edge tiles. Returns per-core idx16 gather tables
    (dma_gather staging layout) and per-edge dst-in-block values [128, nt]."""
    src = np.ascontiguousarray(edge_index[0]).astype(np.int64)
    dst = np.ascontiguousarray(edge_index[1]).astype(np.int64)
    ncn, nb, ncores = cfg.nc_nodes, cfg.nblocks, cfg.ncores
    hcn = ncn // 2

    core = dst // ncn
    local = dst - core * ncn
    blk = local // P
    s_core = src // ncn
    s_local = src - s_core * ncn
    half = s_local // hcn

    # bucket index q per core in device iteration order:
    # pair p -> for h in (0,1) -> member i; full pairs: q = 4p + 2h + i
    pr = blk // 2
    i = blk % 2
    wlast = 1 if nb % 2 == 1 else 2
    wq = np.where(pr == cfg.npairs - 1, wlast, 2)
    q = 4 * pr + half * wq + i
    nq = 4 * (cfg.npairs - 1) + 2 * wlast
    assert nq == nb * 2
    key = core * nq + q

    order = np.argsort(key, kind="stable")
    key_s = key[order]
    counts = np.bincount(key_s, minlength=ncores * nq)
    t_half = max(1, int(math.ceil(counts.max() / P)))
    cfg.t_half = t_half
    nt = nq * t_half

    starts = np.zeros(ncores * nq + 1, np.int64)
    np.cumsum(counts, out=starts[1:])
    rank = np.arange(len(key_s)) - starts[key_s]
    q_s = q[order]
    core_s = core[order]
    ct = q_s * t_half + rank // P
    pp = rank % P
    slot = ct * P + pp
    src16 = (s_core * hcn + s_local - half * hcn).astype(np.int16)[order]
    dloc = (local - blk * P).astype(np.float32)[order]

    idxs, dstbls = [], []
    for c in range(ncores):
        m = core_s == c
        s_arr = np.zeros(nt * P, np.int16)            # pad -> row 0
        d_arr = np.full(nt * P, -1.0, np.float32)     # pad -> -1 (no match)
        s_arr[slot[m]] = src16[m]
        d_arr[slot[m]] = dloc[m]
        st = s_arr.reshape(nt, 8, 16).transpose(2, 0, 1).reshape(16, nt * 8)
        idxs.append(np.ascontiguousarray(np.tile(st, (8, 1))))
        dstbls.append(
            np.ascontiguousarray(d_arr.reshape(nt, P).T.astype(BF16))
        )  # [128, nt] bf16
    return idxs, dstbls


def prep_inputs(x, edge_index, batch_ids, z, params: dict, cfg: Cfg):
    """Build the 8 per-core input maps (layout/casting staging only)."""
    ncn, nb, ncores = cfg.nc_nodes, cfg.nblocks, cfg.ncores
    f_in, h, zd = cfg.f_in, cfg.h, cfg.z_dim
    hc = cfg.h_chunks
    fz = h + zd
    fz_pad = ((fz + P - 1) // P) * P

    idx16s, dstbls = prep_shards(edge_index, cfg)

    # x gather table: fp8, row-permuted to the chunked-AllGather layout:
    # row(g) = half*(N/2) + core(g)*(ncn/2) + local(g) % (ncn/2)
    xpad = np.zeros((cfg.n_nodes, cfg.f_pad), np.float32)
    xpad[:, :f_in] = x
    xpad[:, f_in] = 1.0
    g = np.arange(cfg.n_nodes)
    gc, gl = g // ncn, g % ncn
    hcn = ncn // 2
    ch = gl // hcn
    perm = ch * (cfg.n_nodes // 2) + gc * hcn + (gl - ch * hcn)
    xpad_p = np.empty_like(xpad)
    xpad_p[perm] = xpad
    np_msg = FP8 if mybir.dt.size(cfg.dt_msg) == 1 else BF16
    xgath = np.ascontiguousarray(xpad_p.astype(np_msg))

    batchf_full = batch_ids.astype(np.float32)

    iota128 = np.tile(np.arange(P, dtype=np.float32)[None, :], (P, 1))
    iotagr = np.tile(np.arange(cfg.n_graphs, dtype=np.float32)[None, :], (P, 1))
    ident = np.eye(P, dtype=np.float32)

    def padT(w, rows):  # w [out, in] -> w.T padded to [rows, out]
        wt = np.zeros((rows, w.shape[0]), np.float32)
        wt[: w.shape[1], :] = np.ascontiguousarray(w.T)
        return wt

    def chunk3(wt):  # [K*P_rows, n] -> [P, K, n]
        k = wt.shape[0] // P
        return np.ascontiguousarray(
            wt.reshape(k, P, wt.shape[1]).transpose(1, 0, 2)
        )

    wl1t = padT(params["Wl1"], P).astype(BF16)            # [128, 512]
    wr1t = padT(params["Wr1"], P).astype(BF16)
    wl2t = chunk3(padT(params["Wl2"], h)).astype(BF16)    # [128, 4, 512]
    wr2t = chunk3(padT(params["Wr2"], h)).astype(BF16)
    wf1t = chunk3(padT(params["Wf1"], fz_pad))            # [128, 9, 512] f32
    wf2t = chunk3(padT(params["Wf2"], h))                 # [128, 4, 512] f32
    wpt = chunk3(padT(params["Wp"], h))                   # [128, 4, 1] f32

    brep = lambda b: np.tile(b.astype(np.float32)[None, :], (P, 1))
    bl1r, bl2r = brep(params["bl1"]), brep(params["bl2"])
    bf1r, bf2r = brep(params["bf1"]), brep(params["bf2"])
    bpr = np.tile(params["bp"].astype(np.float32).reshape(1, 1), (P, 1))

    in_maps = []
    for c in range(ncores):
        lo = c * ncn
        xT = np.zeros((P, nb * P), np.float32)
        nrows = min(ncn, cfg.n_nodes - lo)
        xT[:f_in, :nrows] = x[lo : lo + nrows].T
        bfl = np.full(nb * P, -1.0, np.float32)
        bfl[:nrows] = batchf_full[lo : lo + nrows]
        batchf = np.ascontiguousarray(bfl.reshape(nb, P).T)  # node (b,p) at [p,b]
        in_maps.append(
            {
                "xgath": xgath,
                "xT": np.ascontiguousarray(xT.astype(BF16)),
                "idx16": idx16s[c],
                "dstbl": dstbls[c],
                "batchf": batchf,
                "iota128": np.ascontiguousarray(iota128.astype(BF16)),
                "iotagr": iotagr,
                "identb": np.ascontiguousarray(ident.astype(BF16)),
                "identf": ident,
                "wl1t": wl1t,
                "wr1t": wr1t,
                "wl2t": wl2t,
                "wr2t": wr2t,
                "wf1t": wf1t,
                "wf2t": wf2t,
                "wpt": wpt,
                "bl1r": bl1r,
                "bl2r": bl2r,
                "bf1r": bf1r,
                "bf2r": bf2r,
                "bpr": bpr,
                "zin": z.astype(np.float32),
            }
        )
    return in_maps


# ------------------------------- device program -----------------------------
def build_nc(cfg: Cfg):
    ncn, nb, ncores = cfg.nc_nodes, cfg.nblocks, cfg.ncores
    f_in, h, zd, ng = cfg.f_in, cfg.h, cfg.z_dim, cfg.n_graphs
    hc = cfg.h_chunks
    t_half = cfg.t_half
    half_sz = cfg.n_nodes // 2
    hcn = ncn // 2
    nt = nb * 2 * t_half
    fz = h + zd
    fz_pad = ((fz + P - 1) // P) * P
    fzc = fz_pad // P
    f32 = mybir.dt.float32
    bf16 = mybir.dt.bfloat16
    dtm = cfg.dt_msg
    msg_bufs = 3 if mybir.dt.size(dtm) == 1 else 2

    nc = bacc.Bacc(
        "TRN2", target_bir_lowering=False, debug=False, num_devices=ncores,
        num_swdge_queues=cfg.n_queues,
    )
    qctr = [0]

    def next_q():
        qctr[0] += 1
        return qctr[0] % cfg.n_queues

    # ---- I/O ----
    f_pad = cfg.f_pad
    xgath_d = nc.dram_tensor("xgath", [cfg.n_nodes, f_pad], dtm, kind="ExternalInput")
    xT_d = nc.dram_tensor("xT", [P, nb * P], bf16, kind="ExternalInput")
    idx16_d = nc.dram_tensor("idx16", [P, nt * 8], mybir.dt.int16, kind="ExternalInput")
    dstbl_d = nc.dram_tensor("dstbl", [P, nt], bf16, kind="ExternalInput")
    batchf_d = nc.dram_tensor("batchf", [P, nb], f32, kind="ExternalInput")
    iota128_d = nc.dram_tensor("iota128", [P, P], bf16, kind="ExternalInput")
    iotagr_d = nc.dram_tensor("iotagr", [P, ng], f32, kind="ExternalInput")
    identb_d = nc.dram_tensor("identb", [P, P], bf16, kind="ExternalInput")
    identf_d = nc.dram_tensor("identf", [P, P], f32, kind="ExternalInput")
    wl1t_d = nc.dram_tensor("wl1t", [P, h], bf16, kind="ExternalInput")
    wr1t_d = nc.dram_tensor("wr1t", [P, h], bf16, kind="ExternalInput")
    wl2t_d = nc.dram_tensor("wl2t", [P, hc, h], bf16, kind="ExternalInput")
    wr2t_d = nc.dram_tensor("wr2t", [P, hc, h], bf16, kind="ExternalInput")
    wf1t_d = nc.dram_tensor("wf1t", [P, fzc, h], f32, kind="ExternalInput")
    wf2t_d = nc.dram_tensor("wf2t", [P, hc, h], f32, kind="ExternalInput")
    wpt_d = nc.dram_tensor("wpt", [P, hc, 1], f32, kind="ExternalInput")
    bl1r_d = nc.dram_tensor("bl1r", [P, h], f32, kind="ExternalInput")
    bl2r_d = nc.dram_tensor("bl2r", [P, h], f32, kind="ExternalInput")
    bf1r_d = nc.dram_tensor("bf1r", [P, h], f32, kind="ExternalInput")
    bf2r_d = nc.dram_tensor("bf2r", [P, h], f32, kind="ExternalInput")
    bpr_d = nc.dram_tensor("bpr", [P, 1], f32, kind="ExternalInput")
    zin_d = nc.dram_tensor("zin", [ng, zd], f32, kind="ExternalInput")
    out_d = nc.dram_tensor("out", [ng, 1], f32, kind="ExternalOutput")
    if cfg.debug_outputs:
        h1w_dbg = nc.dram_tensor("h1w_dbg", [2 * P, h], f32, kind="ExternalOutput")
        pool_dbg = nc.dram_tensor("pool_dbg", [ng, h], f32, kind="ExternalOutput")
        agg_dbg = nc.dram_tensor("agg_dbg", [P, h], f32, kind="ExternalOutput")

    def rows_of(b):
        return min(P, ncn - b * P)

    with tile.TileContext(nc) as tc:
        with (
            tc.tile_pool(name="const", bufs=1) as cp,
            tc.tile_pool(name="dram", bufs=1, space="DRAM") as dp,
            tc.tile_pool(name="gath", bufs=2) as gp,
            tc.tile_pool(name="oh", bufs=2) as ohp,
            tc.tile_pool(name="work", bufs=2) as wp,
            tc.tile_pool(name="tp", bufs=2) as tp,
            tc.tile_pool(name="ps_agg", bufs=2, space="PSUM") as ps_aggp,
            tc.tile_pool(name="ps_t", bufs=2, space="PSUM") as ps_tp,
            tc.tile_pool(name="ps_o", bufs=2, space="PSUM") as ps_op,
            tc.tile_pool(name="ps_w", bufs=1, space="PSUM") as ps_wp,
            tc.tile_pool(name="ps_pool", bufs=1, space="PSUM") as ps_poolp,
        ):
            # ---- persistent SBUF constants (all pre-cast on host) ----
            idx16 = cp.tile([P, nt * 8], mybir.dt.int16)
            nc.sync.dma_start(idx16[:], idx16_d[:])
            dstbl = cp.tile([P, nt], bf16)
            nc.sync.dma_start(dstbl[:], dstbl_d[:])
            batchf = cp.tile([P, nb], f32)
            nc.sync.dma_start(batchf[:], batchf_d[:])
            iota128 = cp.tile([P, P], bf16)
            nc.sync.dma_start(iota128[:], iota128_d[:])
            iotagr = cp.tile([P, ng], f32)
            nc.sync.dma_start(iotagr[:], iotagr_d[:])
            ident_b = cp.tile([P, P], bf16)
            nc.sync.dma_start(ident_b[:], identb_d[:])
            ident_f = cp.tile([P, P], f32)
            nc.sync.dma_start(ident_f[:], identf_d[:])
            xTs = cp.tile([P, nb * P], bf16)
            nc.sync.dma_start(xTs[:], xT_d[:])
            wl1t = cp.tile([P, h], bf16)
            nc.sync.dma_start(wl1t[:], wl1t_d[:])
            wr1t = cp.tile([P, h], bf16)
            nc.sync.dma_start(wr1t[:], wr1t_d[:])
            wl2t = cp.tile([P, hc, h], bf16)
            nc.sync.dma_start(wl2t[:], wl2t_d[:])
            wr2t = cp.tile([P, hc, h], bf16)
            nc.sync.dma_start(wr2t[:], wr2t_d[:])
            wf1t = cp.tile([P, fzc, h], f32)
            nc.sync.dma_start(wf1t[:], wf1t_d[:])
            wf2t = cp.tile([P, hc, h], f32)
            nc.sync.dma_start(wf2t[:], wf2t_d[:])
            wpt = cp.tile([P, hc, 1], f32)
            nc.sync.dma_start(wpt[:], wpt_d[:])
            bl1r = cp.tile([P, h], f32)
            nc.sync.dma_start(bl1r[:], bl1r_d[:])
            bl2r = cp.tile([P, h], f32)
            nc.sync.dma_start(bl2r[:], bl2r_d[:])
            bf1r = cp.tile([P, h], f32)
            nc.sync.dma_start(bf1r[:], bf1r_d[:])
            bf2r = cp.tile([P, h], f32)
            nc.sync.dma_start(bf2r[:], bf2r_d[:])
            bpr = cp.tile([P, 1], f32)
            nc.sync.dma_start(bpr[:], bpr_d[:])

            inv_all = cp.tile([P, nb], f32)      # 1/max(deg,1) per node

            # ---- internal DRAM ----
            h1w_shard = dp.tile([ncn, h], dtm)               # AG input (local)
            h1w_fa = dp.tile([hcn * ncores, h], dtm, addr_space="Shared")
            h1w_fb = dp.tile([hcn * ncores, h], dtm, addr_space="Shared")
            h1wrb_d = dp.tile([ncn, h], bf16)                # h1@Wr2^T + bl2
            pool_in = dp.tile([ng, h], f32)
            pool_out = dp.tile([ng, h], f32, addr_space="Shared")

            def msg_matmuls(ps, ohs, ms, i, k0, ktot):
                """Accumulate block-member i's t_half tiles from group (oh, m)
                into ps. Returns updated matmul counter."""
                k = k0
                base = i * t_half
                j = 0
                while j < t_half:
                    dr = cfg.use_double_row and j + 1 < t_half
                    w = 2 if dr else 1
                    nc.tensor.matmul(
                        ps[:],
                        lhsT=ohs[:, base + j : base + j + w, :]
                        if w == 2
                        else ohs[:, base + j, :],
                        rhs=ms[:, base + j : base + j + w, :]
                        if w == 2
                        else ms[:, base + j, :],
                        start=(k == k0 and k0 == 0),
                        stop=(k + w == ktot),
                        perf_mode=mybir.MatmulPerfMode.DoubleRow if dr else None,
                    )
                    j += w
                    k += w
                return k

            def mm_per_half():
                if cfg.use_double_row:
                    return t_half  # counted in edge-tiles
                return t_half

            # =================== Layer 1 (+ W2-transform tail) ===============
            for p in range(cfg.npairs):
                members = _pair_members(p, nb)
                w_m = len(members)
                m1s, oh1s = [], []
                for hf in range(2):
                    ct0, ntl = _group_ct0(p, hf, nb, t_half)
                    m1 = gp.tile(
                        [P, ntl, f_pad], dtm, tag="m1",
                        padded_shape=[P, 2 * t_half, f_pad], bufs=msg_bufs,
                    )
                    for g0 in range(0, ntl, cfg.gather_tiles_max):
                        gw = min(cfg.gather_tiles_max, ntl - g0)
                        nc.gpsimd.dma_gather(
                            out_ap=m1[:, g0 : g0 + gw, :],
                            in_ap=xgath_d[hf * half_sz : (hf + 1) * half_sz, :],
                            idxs_ap=idx16[:, (ct0 + g0) * 8 : (ct0 + g0 + gw) * 8],
                            num_idxs=gw * P,
                            num_idxs_reg=gw * P,
                            elem_size=f_pad,
                            queue_num=next_q(),
                        )
                    oh = ohp.tile(
                        [P, ntl, P], dtm, tag="oh",
                        padded_shape=[P, 2 * t_half, P], bufs=msg_bufs,
                    )
                    nc.any.tensor_tensor(
                        out=oh[:],
                        in0=dstbl[:, ct0 : ct0 + ntl]
                        .unsqueeze(2)
                        .to_broadcast([P, ntl, P]),
                        in1=iota128[:].unsqueeze(1).to_broadcast([P, ntl, P]),
                        op=mybir.AluOpType.is_equal,
                    )
                    m1s.append(m1)
                    oh1s.append(oh)
                for i, b in enumerate(members):
                    ps_agg = ps_aggp.tile(
                        [P, f_pad], f32, tag="agg", padded_shape=[P, h]
                    )
                    ktot = 2 * t_half
                    k = msg_matmuls(ps_agg, oh1s[0], m1s[0], i, 0, ktot)
                    msg_matmuls(ps_agg, oh1s[1], m1s[1], i, k, ktot)
                    # degree -> 1/max(cnt,1)
                    cnt = wp.tile([P, 1], f32, tag="cnt")
                    nc.vector.tensor_scalar_max(
                        cnt[:], ps_agg[:, f_in : f_in + 1], 1.0
                    )
                    nc.vector.reciprocal(inv_all[:, b : b + 1], cnt[:])
                    agg = wp.tile([P, P], bf16, tag="agg_sb")
                    nc.vector.tensor_scalar_mul(
                        agg[:], ps_agg[:, :P], inv_all[:, b : b + 1]
                    )
                    ps_t = ps_tp.tile([P, P], bf16, tag="pst", padded_shape=[P, P])
                    nc.tensor.transpose(ps_t[:], agg[:], ident_b[:])
                    aggT = tp.tile([P, P], bf16, tag="aggT")
                    nc.vector.tensor_copy(aggT[:], ps_t[:])
                    ps_o = ps_op.tile([P, h], f32, tag="pso")
                    nc.tensor.matmul(
                        ps_o[:], lhsT=aggT[:], rhs=wl1t[:], start=True, stop=False
                    )
                    nc.tensor.matmul(
                        ps_o[:], lhsT=xTs[:, b * P : (b + 1) * P], rhs=wr1t[:],
                        start=False, stop=True,
                    )
                    o_sb = wp.tile([P, h], f32, tag="o_sb")
                    nc.vector.tensor_tensor(
                        out=o_sb[:], in0=ps_o[:], in1=bl1r[:],
                        op=mybir.AluOpType.add,
                    )
                    ssq = wp.tile([P, 1], f32, tag="ssq")
                    trash = wp.tile([P, h], f32, tag="trash", bufs=1)
                    nc.scalar.activation(
                        trash[:], o_sb[:], mybir.ActivationFunctionType.Square,
                        accum_out=ssq[:],
                    )
                    nrm = wp.tile([P, 1], f32, tag="nrm")
                    nc.scalar.sqrt(nrm[:], ssq[:])
                    nc.vector.tensor_scalar_max(nrm[:], nrm[:], 1e-12)
                    rinv = wp.tile([P, 1], f32, tag="rinv")
                    nc.vector.reciprocal(rinv[:], nrm[:])
                    h1b = wp.tile([P, h], bf16, tag="h1b")
                    nc.scalar.activation(
                        h1b[:], o_sb[:], mybir.ActivationFunctionType.Relu,
                        scale=rinv[:],
                    )
                    # ---- tail: h1W = h1@Wl2^T (fp8, AG table);
                    #            h1Wrb = h1@Wr2^T + bl2 (bf16, local) ----
                    h1T = tp.tile([P, hc, P], bf16, tag="h1T")
                    for kc in range(hc):
                        ps_t2 = ps_tp.tile(
                            [P, P], bf16, tag="pst", padded_shape=[P, P]
                        )
                        nc.tensor.transpose(
                            ps_t2[:], h1b[:, kc * P : (kc + 1) * P], ident_b[:]
                        )
                        nc.vector.tensor_copy(h1T[:, kc, :], ps_t2[:])
                    ps_w = ps_wp.tile([P, h], f32, tag="psw")
                    for kc in range(hc):
                        nc.tensor.matmul(
                            ps_w[:], lhsT=h1T[:, kc, :], rhs=wl2t[:, kc, :],
                            start=(kc == 0), stop=(kc == hc - 1),
                        )
                    h1w_sb = wp.tile([P, h], dtm, tag="h1w_sb")
                    nc.vector.tensor_copy(h1w_sb[:], ps_w[:])
                    r = rows_of(b)
                    nc.sync.dma_start(
                        h1w_shard[b * P : b * P + r, :], h1w_sb[:r, :]
                    )
                    if cfg.debug_outputs and b < 2:
                        nc.sync.dma_start(h1w_dbg[b * P : (b + 1) * P, :], ps_w[:])
                    ps_wr = ps_wp.tile([P, h], f32, tag="psw")
                    for kc in range(hc):
                        nc.tensor.matmul(
                            ps_wr[:], lhsT=h1T[:, kc, :], rhs=wr2t[:, kc, :],
                            start=(kc == 0), stop=(kc == hc - 1),
                        )
                    h1wrb_sb = wp.tile([P, h], bf16, tag="h1wrb_sb")
                    nc.vector.tensor_tensor(
                        out=h1wrb_sb[:], in0=ps_wr[:], in1=bl2r[:],
                        op=mybir.AluOpType.add,
                    )
                    nc.sync.dma_start(
                        h1wrb_d[b * P : b * P + r, :], h1wrb_sb[:r, :]
                    )

            # =================== AllGather h1W (2 chunks) ===================
            if ncores > 1:
                nc.gpsimd.collective_compute(
                    "AllGather",
                    mybir.AluOpType.bypass,
                    replica_groups=[list(range(ncores))],
                    ins=[h1w_shard[:hcn, :].opt()],
                    outs=[h1w_fa.opt()],
                )
                nc.gpsimd.collective_compute(
                    "AllGather",
                    mybir.AluOpType.bypass,
                    replica_groups=[list(range(ncores))],
                    ins=[h1w_shard[hcn:, :].opt()],
                    outs=[h1w_fb.opt()],
                )
            else:
                nc.sync.dma_start(h1w_fa[:hcn, :], h1w_shard[:hcn, :])
                nc.sync.dma_start(h1w_fb[:hcn, :], h1w_shard[hcn:, :])

            # =================== Layer 2 (+ fused graph pooling) =============
            ps_pool = ps_poolp.tile([ng, h], f32)
            for p in range(cfg.npairs):
                members = _pair_members(p, nb)
                m2s, oh2s = [], []
                for hf in range(2):
                    ct0, ntl = _group_ct0(p, hf, nb, t_half)
                    m2 = gp.tile(
                        [P, ntl, h], dtm, tag="m2",
                        padded_shape=[P, 2 * t_half, h], bufs=msg_bufs,
                    )
                    for g0 in range(0, ntl, cfg.gather_tiles_max):
                        gw = min(cfg.gather_tiles_max, ntl - g0)
                        nc.gpsimd.dma_gather(
                            out_ap=m2[:, g0 : g0 + gw, :],
                            in_ap=(h1w_fa if hf == 0 else h1w_fb)[:],
                            idxs_ap=idx16[:, (ct0 + g0) * 8 : (ct0 + g0 + gw) * 8],
                            num_idxs=gw * P,
                            num_idxs_reg=gw * P,
                            elem_size=h,
                            queue_num=next_q(),
                        )
                    oh = ohp.tile(
                        [P, ntl, P], dtm, tag="oh",
                        padded_shape=[P, 2 * t_half, P], bufs=msg_bufs,
                    )
                    nc.any.tensor_tensor(
                        out=oh[:],
                        in0=dstbl[:, ct0 : ct0 + ntl]
                        .unsqueeze(2)
                        .to_broadcast([P, ntl, P]),
                        in1=iota128[:].unsqueeze(1).to_broadcast([P, ntl, P]),
                        op=mybir.AluOpType.is_equal,
                    )
                    m2s.append(m2)
                    oh2s.append(oh)
                for i, b in enumerate(members):
                    r = rows_of(b)
                    h1wrb_sb = wp.tile([P, h], bf16, tag="h1wrb_in")
                    if r < P:
                        nc.gpsimd.memset(h1wrb_sb[:], 0.0)
                    nc.sync.dma_start(
                        h1wrb_sb[:r, :], h1wrb_d[b * P : b * P + r, :]
                    )
                    ps_agg = ps_aggp.tile([P, h], f32, tag="agg")
                    ktot = 2 * t_half
                    k = msg_matmuls(ps_agg, oh2s[0], m2s[0], i, 0, ktot)
                    msg_matmuls(ps_agg, oh2s[1], m2s[1], i, k, ktot)
                    o2 = wp.tile([P, h], f32, tag="o2")
                    nc.vector.tensor_scalar_mul(
                        o2[:], ps_agg[:], inv_all[:, b : b + 1]
                    )
                    if cfg.debug_outputs and b == 0:
                        nc.sync.dma_start(agg_dbg[:], o2[:])
                    o2b = wp.tile([P, h], f32, tag="o2b")
                    nc.vector.tensor_tensor(
                        out=o2b[:], in0=o2[:], in1=h1wrb_sb[:],
                        op=mybir.AluOpType.add,
                    )
                    ssq = wp.tile([P, 1], f32, tag="ssq")
                    trash = wp.tile([P, h], f32, tag="trash", bufs=1)
                    nc.scalar.activation(
                        trash[:], o2b[:], mybir.ActivationFunctionType.Square,
                        accum_out=ssq[:],
                    )
                    nrm = wp.tile([P, 1], f32, tag="nrm")
                    nc.scalar.sqrt(nrm[:], ssq[:])
                    nc.vector.tensor_scalar_max(nrm[:], nrm[:], 1e-12)
                    rinv = wp.tile([P, 1], f32, tag="rinv")
                    nc.vector.reciprocal(rinv[:], nrm[:])
                    h2b = wp.tile([P, h], bf16, tag="h2b")
                    nc.scalar.activation(
                        h2b[:], o2b[:], mybir.ActivationFunctionType.Relu,
                        scale=rinv[:],
                    )
                    G = wp.tile([P, ng], bf16, tag="G")
                    nc.vector.tensor_tensor(
                        out=G[:],
                        in0=batchf[:, b : b + 1].to_broadcast([P, ng]),
                        in1=iotagr[:],
                        op=mybir.AluOpType.is_equal,
                    )
                    nc.tensor.matmul(
                        ps_pool[:],
                        lhsT=G[:],
                        rhs=h2b[:],
                        start=(b == 0),
                        stop=(b == nb - 1),
                    )

            # =================== pool AllReduce ===================
            pool_sb = wp.tile([ng, h], f32, tag="pool_sb")
            nc.vector.tensor_copy(pool_sb[:], ps_pool[:])
            nc.sync.dma_start(pool_in[:], pool_sb[:])
            if ncores > 1:
                nc.gpsimd.collective_compute(
                    "AllReduce",
                    mybir.AluOpType.add,
                    replica_groups=[list(range(ncores))],
                    ins=[pool_in.opt()],
                    outs=[pool_out.opt()],
                )
            else:
                nc.sync.dma_start(pool_out[:], pool_in[:])
            if cfg.debug_outputs:
                pdb_sb = wp.tile([ng, h], f32, tag="pdb_sb")
                nc.sync.dma_start(pdb_sb[:], pool_out[:])
                nc.sync.dma_start(pool_dbg[:], pdb_sb[:])

            # =================== MLP head (replicated) ===================
            f_sb = cp.tile([ng, fz_pad], f32)
            if fz_pad > fz:
                nc.gpsimd.memset(f_sb[:, fz:], 0.0)
            nc.sync.dma_start(f_sb[:, :h], pool_out[:])
            nc.sync.dma_start(f_sb[:, h : h + zd], zin_d[:])

            def dense_small(in_sb, w_sb, nchunks, n_out, bias, relu):
                ps = ps_op.tile([ng, n_out], f32, tag="pso", padded_shape=[P, h])
                for k in range(nchunks):
                    ps_t = ps_tp.tile(
                        [P, ng], f32, tag="pst", padded_shape=[P, P]
                    )
                    nc.tensor.transpose(
                        ps_t[:], in_sb[:, k * P : (k + 1) * P], ident_f[:ng, :ng]
                    )
                    fT = tp.tile([P, ng], f32, tag="fT")
                    nc.vector.tensor_copy(fT[:], ps_t[:])
                    nc.tensor.matmul(
                        ps[:],
                        lhsT=fT[:],
                        rhs=w_sb[:, k, :],
                        start=(k == 0),
                        stop=(k == nchunks - 1),
                    )
                g_sb = cp.tile([ng, n_out], f32, name=f"g_{relu}_{n_out}_{nchunks}")
                if bias is not None:
                    nc.vector.tensor_tensor(
                        out=g_sb[:], in0=ps[:], in1=bias[:ng, :n_out],
                        op=mybir.AluOpType.add,
                    )
                else:
                    nc.vector.tensor_copy(g_sb[:], ps[:])
                if relu:
                    nc.scalar.activation(
                        g_sb[:], g_sb[:], mybir.ActivationFunctionType.Relu
                    )
                return g_sb

            g1 = dense_small(f_sb, wf1t, fzc, h, bf1r, True)
            g2 = dense_small(g1, wf2t, hc, h, bf2r, True)
            g3 = dense_small(g2, wpt, hc, 1, bpr, False)
            res = cp.tile([ng, 1], f32)
            nc.scalar.activation(
                res[:], g3[:], mybir.ActivationFunctionType.Sigmoid
            )
            nc.scalar.activation(res[:], res[:], mybir.ActivationFunctionType.Ln)
            nc.scalar.mul(res[:], res[:], -1.0)
            nc.sync.dma_start(out_d[:], res[:])

    nc.compile()
    return nc


# ------------------------------- entry point --------------------------------
def kernel(x, edge_index, batch_ids, z, Wl1, bl1, Wr1, Wl2, bl2, Wr2,
           Wf1, bf1, Wf2, bf2, Wp, bp, _cfg: Cfg | None = None):
    import os

    from concourse.bass_utils import run_bass_kernel_spmd

    cfg = _cfg or Cfg()
    if _cfg is None:  # env overrides for quick A/B during tuning
        if os.environ.get("KCFG_DT") == "bf16":
            cfg.dt_msg = mybir.dt.bfloat16
        if os.environ.get("KCFG_DR") == "0":
            cfg.use_double_row = False
        if os.environ.get("KCFG_GMAX"):
            cfg.gather_tiles_max = int(os.environ["KCFG_GMAX"])
    params = dict(Wl1=Wl1, bl1=bl1, Wr1=Wr1, Wl2=Wl2, bl2=bl2, Wr2=Wr2,
                  Wf1=Wf1, bf1=bf1, Wf2=Wf2, bf2=bf2, Wp=Wp, bp=bp)
    x = np.asarray(x, np.float32)
    z = np.asarray(z, np.float32)
    edge_index = np.asarray(edge_index)
    batch_ids = np.asarray(batch_ids)
    params = {k: np.asarray(v, np.float32) for k, v in params.items()}

    in_maps = prep_inputs(x, edge_index, batch_ids, z, params, cfg)
    nc = build_nc(cfg)
    res = run_bass_kernel_spmd(
        nc,
        in_maps,
        core_ids=list(range(cfg.ncores)),
        trace=cfg.trace or bool(os.environ.get("BASS_TRACE")),
    )
    LAST_RUN_INFO["exec_time_ns"] = res.exec_time_ns
    LAST_RUN_INFO["res"] = res
    LAST_RUN_INFO["results"] = res.results if cfg.debug_outputs else None
    return np.asarray(res.results[0]["out"], np.float32)
